# revision 41
# baseline (speedup 1.0000x reference)
import os
from contextlib import ExitStack

import numpy as np
import ml_dtypes

import concourse.bass as bass
import concourse.tile as tile
from concourse import bacc, mybir
from concourse import bass_utils

G = 8          # games / cores
D_IN = 1024    # input dim
F = 80         # hidden features
A = 18         # actions
TILE_N = 512   # rows per device tile (one PSUM bank of fp32)
KP = 128       # contraction chunk (SBUF partitions)
NK = D_IN // KP

BF16 = ml_dtypes.bfloat16

_NC_CACHE: dict[int, object] = {}
LAST_RESULTS = None


def _build_nc(R: int, LV: int):
    n_tiles = R // TILE_N
    n_main = n_tiles - 1
    W = n_main * TILE_N + LV
    nc = bacc.Bacc(
        "TRN2",
        target_bir_lowering=False,
        debug=False,
        enable_asserts=False,
        num_devices=G,
    )
    xt = nc.dram_tensor(
        "xt", [max(n_main, 1), KP, NK * TILE_N], mybir.dt.bfloat16, kind="ExternalInput"
    ).ap()
    xl = nc.dram_tensor(
        "xl", [KP, NK * LV], mybir.dt.bfloat16, kind="ExternalInput"
    ).ap()
    w1 = nc.dram_tensor("w1", [KP, NK * F], mybir.dt.bfloat16, kind="ExternalInput").ap()
    w2 = nc.dram_tensor("w2", [F, F], mybir.dt.bfloat16, kind="ExternalInput").ap()
    w3 = nc.dram_tensor("w3", [F, A], mybir.dt.bfloat16, kind="ExternalInput").ap()
    b1 = nc.dram_tensor("b1", [F, 1], mybir.dt.float32, kind="ExternalInput").ap()
    b2 = nc.dram_tensor("b2", [F, 1], mybir.dt.float32, kind="ExternalInput").ap()
    b3 = nc.dram_tensor("b3", [A, 1], mybir.dt.float32, kind="ExternalInput").ap()
    qt = nc.dram_tensor("qt", [A, W], mybir.dt.float32, kind="ExternalOutput").ap()

    with tile.TileContext(nc) as tc:
        with ExitStack() as ctx:
            const = ctx.enter_context(tc.tile_pool(name="const", bufs=1))
            xp = ctx.enter_context(tc.tile_pool(name="xp", bufs=max(n_main, 1)))
            xlp = ctx.enter_context(tc.tile_pool(name="xlp", bufs=1))
            h1p = ctx.enter_context(tc.tile_pool(name="h1p", bufs=3))
            h2p = ctx.enter_context(tc.tile_pool(name="h2p", bufs=3))
            qp = ctx.enter_context(tc.tile_pool(name="qp", bufs=1))
            pp1 = ctx.enter_context(tc.tile_pool(name="pp1", bufs=3, space="PSUM"))
            pp2 = ctx.enter_context(tc.tile_pool(name="pp2", bufs=3, space="PSUM"))
            pp3 = ctx.enter_context(tc.tile_pool(name="pp3", bufs=2, space="PSUM"))

            w1_s = const.tile([KP, NK * F], mybir.dt.bfloat16)
            w2_s = const.tile([F, F], mybir.dt.bfloat16)
            w3_s = const.tile([F, A], mybir.dt.bfloat16)
            b1_s = const.tile([F, 1], mybir.dt.float32)
            b2_s = const.tile([F, 1], mybir.dt.float32)
            b3_s = const.tile([A, 1], mybir.dt.float32)
            q_big = qp.tile([A, W], mybir.dt.float32)
            nc.scalar.dma_start(w1_s[:], w1[:])
            nc.scalar.dma_start(w2_s[:], w2[:])
            nc.scalar.dma_start(w3_s[:], w3[:])
            nc.scalar.dma_start(b1_s[:], b1[:])
            nc.scalar.dma_start(b2_s[:], b2[:])
            nc.scalar.dma_start(b3_s[:], b3[:])

            p1 = [None] * n_tiles
            p2 = [None] * n_tiles
            h1 = [None] * n_tiles

            def tw(b):
                return TILE_N if b < n_main else LV

            def stage1(b):
                w = tw(b)
                if b < n_main:
                    x_s = xp.tile([KP, NK * TILE_N], mybir.dt.bfloat16, name="x_s")
                    src = xt[b]
                else:
                    x_s = xlp.tile([KP, NK * LV], mybir.dt.bfloat16, name="xl_s")
                    src = xl
                nsplit = 4 if b == 0 else 2
                qw = NK * w // nsplit
                for j in range(nsplit):
                    nc.sync.dma_start(
                        x_s[:, j * qw:(j + 1) * qw], src[:, j * qw:(j + 1) * qw]
                    )
                p = pp1.tile([F, TILE_N], mybir.dt.float32, name="p1_t")
                for k in range(NK):
                    nc.tensor.matmul(
                        p[:, :w],
                        lhsT=w1_s[:, k * F:(k + 1) * F],
                        rhs=x_s[:, k * w:(k + 1) * w],
                        start=(k == 0),
                        stop=(k == NK - 1),
                    )
                p1[b] = p

            def stage2(b):
                w = tw(b)
                h = h1p.tile([F, TILE_N], mybir.dt.bfloat16, name="h1_t")
                nc.scalar.activation(
                    h[:, :w], p1[b][:, :w], mybir.ActivationFunctionType.Relu,
                    bias=b1_s[:],
                )
                h1[b] = h
                p = pp2.tile([F, TILE_N], mybir.dt.float32, name="p2_t")
                nc.tensor.matmul(
                    p[:, :w], lhsT=w2_s[:], rhs=h[:, :w], start=True, stop=True
                )
                p2[b] = p

            def stage3(b):
                w = tw(b)
                h = h2p.tile([F, TILE_N], mybir.dt.bfloat16, name="h2_t")
                nc.scalar.activation(
                    h[:, :w], p2[b][:, :w], mybir.ActivationFunctionType.Relu,
                    bias=b2_s[:],
                )
                p = pp3.tile([A, TILE_N], mybir.dt.float32, name="p3_t")
                nc.tensor.matmul(
                    p[:, :w], lhsT=w3_s[:], rhs=h[:, :w], start=True, stop=True
                )
                nc.vector.tensor_scalar_add(
                    q_big[:, b * TILE_N:b * TILE_N + w], p[:, :w], b3_s[:]
                )

            for b in range(n_tiles):
                stage1(b)
                if b >= 1:
                    stage2(b - 1)
                if b >= 2:
                    stage3(b - 2)
            stage2(n_tiles - 1)
            if n_tiles >= 2:
                stage3(n_tiles - 2)
            stage3(n_tiles - 1)
            bnds = [0, W // 4, W // 2, 3 * W // 4, W]
            for j in range(4):
                eng = nc.sync if j % 2 == 0 else nc.gpsimd
                eng.dma_start(
                    qt[:, bnds[j]:bnds[j + 1]], q_big[:, bnds[j]:bnds[j + 1]]
                )

    nc.compile()
    return nc


def kernel(state, idx, W1, b1, W2, b2, W3, b3):
    global LAST_RESULTS
    state = np.ascontiguousarray(np.asarray(state, dtype=np.float32))
    idx_i = np.asarray(idx).astype(np.int64)
    W1 = np.asarray(W1, dtype=np.float32)
    b1 = np.asarray(b1, dtype=np.float32)
    W2 = np.asarray(W2, dtype=np.float32)
    b2 = np.asarray(b2, dtype=np.float32)
    W3 = np.asarray(W3, dtype=np.float32)
    b3 = np.asarray(b3, dtype=np.float32)

    B = state.shape[0]
    counts = np.bincount(idx_i, minlength=G)
    assert counts.size == G, "idx out of range"
    if np.all(idx_i[:-1] <= idx_i[1:]):
        order = np.arange(B)
    else:
        order = np.argsort(idx_i, kind="stable")
    offs = np.concatenate([[0], np.cumsum(counts)])
    mx = int(counts.max())
    R = max(TILE_N, -(-mx // TILE_N) * TILE_N)
    n_tiles = R // TILE_N
    n_main = n_tiles - 1
    lv = mx - n_main * TILE_N
    LV = min(TILE_N, -(-lv // 64) * 64)
    W = n_main * TILE_N + LV

    nc = _NC_CACHE.get((R, LV))
    if nc is None:
        nc = _build_nc(R, LV)
        _NC_CACHE[(R, LV)] = nc

    w2_bf = W2.astype(BF16)
    b2_col = b2.reshape(F, 1)
    in_maps = []
    for c in range(G):
        rows = state[order[offs[c]:offs[c + 1]]]
        xT = np.zeros((D_IN, W), dtype=BF16)
        xT[:, :rows.shape[0]] = rows.T.astype(BF16)
        # exact SBUF image per tile -> one DMA, long contiguous runs
        xm = xT[:, :n_main * TILE_N]
        xt = np.ascontiguousarray(
            xm.reshape(NK, KP, n_main, TILE_N)
            .transpose(2, 1, 0, 3)
            .reshape(max(n_main, 1), KP, NK * TILE_N)
        ) if n_main else np.zeros((1, KP, NK * TILE_N), dtype=BF16)
        xlast = np.ascontiguousarray(
            xT[:, n_main * TILE_N:]
            .reshape(NK, KP, LV)
            .transpose(1, 0, 2)
            .reshape(KP, NK * LV)
        )
        w1p = (
            W1[c].reshape(NK, KP, F).transpose(1, 0, 2).reshape(KP, NK * F).astype(BF16)
        )
        in_maps.append({
            "xt": xt,
            "xl": xlast,
            "w1": np.ascontiguousarray(w1p),
            "w2": w2_bf,
            "w3": W3[c].astype(BF16),
            "b1": np.ascontiguousarray(b1[c].reshape(F, 1)),
            "b2": b2_col,
            "b3": np.ascontiguousarray(b3[c].reshape(A, 1)),
        })

    trace = bool(os.environ.get("KERNEL_TRACE"))
    res = bass_utils.run_bass_kernel_spmd(
        nc, in_maps, core_ids=list(range(G)), trace=trace
    )
    LAST_RESULTS = res

    q = np.empty((B, A), dtype=np.float32)
    for c in range(G):
        qtc = np.asarray(res.results[c]["qt"])  # [A, W]
        q[order[offs[c]:offs[c + 1]]] = qtc[:, :counts[c]].T
    return q


# revision 45
# speedup vs baseline: 1.1215x; 1.1215x over previous
import os
from contextlib import ExitStack

import numpy as np
import ml_dtypes

import concourse.bass as bass
import concourse.tile as tile
from concourse import bacc, mybir
from concourse import bass_utils

G = 8          # games / cores
D_IN = 1024    # input dim
F = 80         # hidden features
A = 18         # actions
TILE_N = 512   # rows per device tile (one PSUM bank of fp32)
KP = 128       # contraction chunk (SBUF partitions)
NK = D_IN // KP

BF16 = ml_dtypes.bfloat16

_NC_CACHE: dict[int, object] = {}
LAST_RESULTS = None


def _build_nc(R: int, LV: int):
    n_tiles = R // TILE_N
    n_main = n_tiles - 1
    W = n_main * TILE_N + LV
    nc = bacc.Bacc(
        "TRN2",
        target_bir_lowering=False,
        debug=False,
        enable_asserts=False,
        num_devices=G,
    )
    xt = nc.dram_tensor(
        "xt", [max(n_main, 1), KP, NK * TILE_N], mybir.dt.bfloat16, kind="ExternalInput"
    ).ap()
    xl = nc.dram_tensor(
        "xl", [KP, NK * LV], mybir.dt.bfloat16, kind="ExternalInput"
    ).ap()
    w1 = nc.dram_tensor("w1", [KP, NK * F], mybir.dt.bfloat16, kind="ExternalInput").ap()
    w2 = nc.dram_tensor("w2", [F, F], mybir.dt.bfloat16, kind="ExternalInput").ap()
    w3 = nc.dram_tensor("w3", [F, A], mybir.dt.bfloat16, kind="ExternalInput").ap()
    b1 = nc.dram_tensor("b1", [F, 1], mybir.dt.float32, kind="ExternalInput").ap()
    b2 = nc.dram_tensor("b2", [F, 1], mybir.dt.float32, kind="ExternalInput").ap()
    b3 = nc.dram_tensor("b3", [A, 1], mybir.dt.float32, kind="ExternalInput").ap()
    qt = nc.dram_tensor("qt", [A, W], mybir.dt.float32, kind="ExternalOutput").ap()

    with tile.TileContext(nc) as tc:
        with ExitStack() as ctx:
            const = ctx.enter_context(tc.tile_pool(name="const", bufs=1))
            xp = ctx.enter_context(tc.tile_pool(name="xp", bufs=max(n_main, 1)))
            xlp = ctx.enter_context(tc.tile_pool(name="xlp", bufs=1))
            h1p = ctx.enter_context(tc.tile_pool(name="h1p", bufs=3))
            h2p = ctx.enter_context(tc.tile_pool(name="h2p", bufs=3))
            qp = ctx.enter_context(tc.tile_pool(name="qp", bufs=3))
            pp1 = ctx.enter_context(tc.tile_pool(name="pp1", bufs=3, space="PSUM"))
            pp2 = ctx.enter_context(tc.tile_pool(name="pp2", bufs=3, space="PSUM"))
            pp3 = ctx.enter_context(tc.tile_pool(name="pp3", bufs=2, space="PSUM"))

            w1_s = const.tile([KP, NK * F], mybir.dt.bfloat16)
            w2_s = const.tile([F, F], mybir.dt.bfloat16)
            w3_s = const.tile([F, A], mybir.dt.bfloat16)
            b1_s = const.tile([F, 1], mybir.dt.float32)
            b2_s = const.tile([F, 1], mybir.dt.float32)
            b3_s = const.tile([A, 1], mybir.dt.float32)
            nc.scalar.dma_start(w1_s[:], w1[:])
            nc.scalar.dma_start(w2_s[:], w2[:])
            nc.scalar.dma_start(w3_s[:], w3[:])
            nc.scalar.dma_start(b1_s[:], b1[:])
            nc.scalar.dma_start(b2_s[:], b2[:])
            nc.scalar.dma_start(b3_s[:], b3[:])

            p1 = [None] * n_tiles
            p2 = [None] * n_tiles
            h1 = [None] * n_tiles

            def tw(b):
                return TILE_N if b < n_main else LV

            def stage1(b):
                w = tw(b)
                if b < n_main:
                    x_s = xp.tile([KP, NK * TILE_N], mybir.dt.bfloat16, name="x_s")
                    src = xt[b]
                else:
                    x_s = xlp.tile([KP, NK * LV], mybir.dt.bfloat16, name="xl_s")
                    src = xl
                nsplit = 4 if b == 0 else 2
                qw = NK * w // nsplit
                for j in range(nsplit):
                    nc.sync.dma_start(
                        x_s[:, j * qw:(j + 1) * qw], src[:, j * qw:(j + 1) * qw]
                    )
                p = pp1.tile([F, TILE_N], mybir.dt.float32, name="p1_t")
                for k in range(NK):
                    nc.tensor.matmul(
                        p[:, :w],
                        lhsT=w1_s[:, k * F:(k + 1) * F],
                        rhs=x_s[:, k * w:(k + 1) * w],
                        start=(k == 0),
                        stop=(k == NK - 1),
                    )
                p1[b] = p

            def stage2(b):
                w = tw(b)
                h = h1p.tile([F, TILE_N], mybir.dt.bfloat16, name="h1_t")
                nc.scalar.activation(
                    h[:, :w], p1[b][:, :w], mybir.ActivationFunctionType.Relu,
                    bias=b1_s[:],
                )
                h1[b] = h
                p = pp2.tile([F, TILE_N], mybir.dt.float32, name="p2_t")
                nc.tensor.matmul(
                    p[:, :w], lhsT=w2_s[:], rhs=h[:, :w], start=True, stop=True
                )
                p2[b] = p

            def stage3(b):
                w = tw(b)
                h = h2p.tile([F, TILE_N], mybir.dt.bfloat16, name="h2_t")
                nc.scalar.activation(
                    h[:, :w], p2[b][:, :w], mybir.ActivationFunctionType.Relu,
                    bias=b2_s[:],
                )
                p = pp3.tile([A, TILE_N], mybir.dt.float32, name="p3_t")
                nc.tensor.matmul(
                    p[:, :w], lhsT=w3_s[:], rhs=h[:, :w], start=True, stop=True
                )
                q_t = qp.tile([A, TILE_N], mybir.dt.float32, name="q_t")
                nc.vector.tensor_scalar_add(q_t[:, :w], p[:, :w], b3_s[:])
                nc.gpsimd.dma_start(
                    qt[:, b * TILE_N:b * TILE_N + w], q_t[:, :w]
                )

            for b in range(n_tiles):
                stage1(b)
                if b >= 1:
                    stage2(b - 1)
                if b >= 2:
                    stage3(b - 2)
            stage2(n_tiles - 1)
            if n_tiles >= 2:
                stage3(n_tiles - 2)
            stage3(n_tiles - 1)

    nc.compile()
    return nc


def kernel(state, idx, W1, b1, W2, b2, W3, b3):
    global LAST_RESULTS
    state = np.ascontiguousarray(np.asarray(state, dtype=np.float32))
    idx_i = np.asarray(idx).astype(np.int64)
    W1 = np.asarray(W1, dtype=np.float32)
    b1 = np.asarray(b1, dtype=np.float32)
    W2 = np.asarray(W2, dtype=np.float32)
    b2 = np.asarray(b2, dtype=np.float32)
    W3 = np.asarray(W3, dtype=np.float32)
    b3 = np.asarray(b3, dtype=np.float32)

    B = state.shape[0]
    counts = np.bincount(idx_i, minlength=G)
    assert counts.size == G, "idx out of range"
    if np.all(idx_i[:-1] <= idx_i[1:]):
        order = np.arange(B)
    else:
        order = np.argsort(idx_i, kind="stable")
    offs = np.concatenate([[0], np.cumsum(counts)])
    mx = int(counts.max())
    R = max(TILE_N, -(-mx // TILE_N) * TILE_N)
    n_tiles = R // TILE_N
    n_main = n_tiles - 1
    lv = mx - n_main * TILE_N
    LV = min(TILE_N, -(-lv // 64) * 64)
    W = n_main * TILE_N + LV

    nc = _NC_CACHE.get((R, LV))
    if nc is None:
        nc = _build_nc(R, LV)
        _NC_CACHE[(R, LV)] = nc

    w2_bf = W2.astype(BF16)
    b2_col = b2.reshape(F, 1)
    in_maps = []
    for c in range(G):
        rows = state[order[offs[c]:offs[c + 1]]]
        xT = np.zeros((D_IN, W), dtype=BF16)
        xT[:, :rows.shape[0]] = rows.T.astype(BF16)
        # exact SBUF image per tile -> one DMA, long contiguous runs
        xm = xT[:, :n_main * TILE_N]
        xt = np.ascontiguousarray(
            xm.reshape(NK, KP, n_main, TILE_N)
            .transpose(2, 1, 0, 3)
            .reshape(max(n_main, 1), KP, NK * TILE_N)
        ) if n_main else np.zeros((1, KP, NK * TILE_N), dtype=BF16)
        xlast = np.ascontiguousarray(
            xT[:, n_main * TILE_N:]
            .reshape(NK, KP, LV)
            .transpose(1, 0, 2)
            .reshape(KP, NK * LV)
        )
        w1p = (
            W1[c].reshape(NK, KP, F).transpose(1, 0, 2).reshape(KP, NK * F).astype(BF16)
        )
        in_maps.append({
            "xt": xt,
            "xl": xlast,
            "w1": np.ascontiguousarray(w1p),
            "w2": w2_bf,
            "w3": W3[c].astype(BF16),
            "b1": np.ascontiguousarray(b1[c].reshape(F, 1)),
            "b2": b2_col,
            "b3": np.ascontiguousarray(b3[c].reshape(A, 1)),
        })

    trace = bool(os.environ.get("KERNEL_TRACE"))
    res = bass_utils.run_bass_kernel_spmd(
        nc, in_maps, core_ids=list(range(G)), trace=trace
    )
    LAST_RESULTS = res

    q = np.empty((B, A), dtype=np.float32)
    for c in range(G):
        qtc = np.asarray(res.results[c]["qt"])  # [A, W]
        q[order[offs[c]:offs[c + 1]]] = qtc[:, :counts[c]].T
    return q
